# revision 1
# baseline (speedup 1.0000x reference)
"""Trainium2 Bass kernel for DeTrAttention (dense transformer MHA block).

Full op: out = softmax((q@Wq+bq)(k@Wk+bk)^T / sqrt(64)) (v@Wv+bv) @ Wo + bo
Shapes: q,k,v [B=2, S=2048, H=1024], NH=16 heads, HD=64.

Sharding (8 cores): data-parallel over batch (2 groups of 4 cores); within a
group each core owns a block of 512 query rows end-to-end (all heads), and
redundantly computes the K/V projections for its batch's full sequence.  No
collectives needed; every core writes a disjoint slice of the output.

On-chip layout strategy (avoids every on-chip transpose):
  - host passes q^T, k^T, v^T slices ([H, tokens], feature-major)
  - qp^T, kp^T computed with W stationary -> [head-major, tokens]
  - vp computed token-major via lhsT = v^T tiles
  - scores^T[kt, qt] = kp_h-slice.T @ qp_h  (contraction over head dim)
  - exp on scalar engine (no max subtraction: |scores| <= ~8 so exp is safe)
  - ctx^T[d, qt] accumulated over kt blocks with lhsT = vp (token-major);
    an appended ones-column in vp yields the softmax denominator Z in row 64
  - normalize via reciprocal + K=1 broadcast matmul + DVE multiply
  - out^T[o, t] = Wo-stationary matmul over ctx^T blocks; biases are applied
    with K=1 matmuls against a ones vector
Projections/scores run in float32r (fp32 data, ~16x better accuracy than
bf16, full PE rate for free dims >= 256); attention probabilities and V are
bf16 (end-to-end rel err ~2e-3).  DMAs are batched into multi-MB 3D transfers
(HWDGE descriptor generation costs ~625ns per dma_start regardless of size).
"""

import contextlib

import numpy as np

import concourse.bass as bass
import concourse.tile as tile
from concourse import bacc, mybir
from concourse.bass_utils import run_bass_kernel_spmd

F32 = mybir.dt.float32
F32R = mybir.dt.float32r
BF16 = mybir.dt.bfloat16

B, S, H, NH = 2, 2048, 1024, 16
HD = H // NH  # 64
N_CORES = 8
GROUPS = B  # batch groups
CPG = N_CORES // GROUPS  # cores per group (4)
SQ = S // CPG  # query rows per core (512)


def build_nc(s=S, h=H, nh=NH, sq=SQ, reps=0, upto=3, exp_sbuf=True, v2=False,
             ag=False, sreps=1):
    """Build the per-core Bass program (same program on all 8 cores).

    reps > 0 wraps the body in a hardware For_i loop (timing vehicle).
    sreps > 1 statically unrolls the body (timing vehicle that allows
    collectives, which cannot live inside control flow).
    """
    hd = h // nh
    assert hd == 64
    KB = h // 128          # contraction blocks over h_in
    KTB = s // 128         # key-token 128-blocks
    WN = min(512, s)       # free-dim block for kpT/vp projections
    NKT = s // WN          # kt 512-blocks
    TPW = WN // 128        # token 128-blocks per 512-block
    MB = h // 128          # ho 128-blocks total
    MH = MB // 2           # ho blocks per half
    HPH = nh // 2          # heads per half
    WH = h // 2            # ho columns per half
    assert WH <= 512 and sq <= 512

    nc = bacc.Bacc("TRN2", target_bir_lowering=False, debug=False,
                   num_devices=8 if ag else 1)

    SL = s // 4 if ag else s   # local K/V token span (4-rank AllGather)
    qT = nc.dram_tensor("qT", [h, sq], F32R, kind="ExternalInput").ap()
    kT = nc.dram_tensor("kT", [h, SL], F32R, kind="ExternalInput").ap()
    vT = nc.dram_tensor("vT", [h, SL], F32R, kind="ExternalInput").ap()
    Wq = nc.dram_tensor("Wq", [h, h], F32R, kind="ExternalInput").ap()
    Wk = nc.dram_tensor("Wk", [h, h], F32R, kind="ExternalInput").ap()
    Wv = nc.dram_tensor("Wv", [h, h], F32R, kind="ExternalInput").ap()
    Wo = nc.dram_tensor("Wo", [h, h], F32R, kind="ExternalInput").ap()
    bq = nc.dram_tensor("bq", [1, h], F32R, kind="ExternalInput").ap()
    bk = nc.dram_tensor("bk", [1, h], F32R, kind="ExternalInput").ap()
    bv = nc.dram_tensor("bv", [1, h], F32R, kind="ExternalInput").ap()
    bo = nc.dram_tensor("bo", [1, h], F32R, kind="ExternalInput").ap()
    outT = nc.dram_tensor("outT", [h, sq], F32, kind="ExternalOutput").ap()

    # [p, kb, cols] views (partition-major) so whole weights load in one DMA
    qT_p = qT.rearrange("(kb p) t -> p kb t", p=128)
    kT_p = kT.rearrange("(kb p) t -> p kb t", p=128)
    vT_p = vT.rearrange("(kb p) t -> p kb t", p=128)
    Wq_p = Wq.rearrange("(kb p) o -> p kb o", p=128)
    Wk_p = Wk.rearrange("(kb p) o -> p kb o", p=128)
    Wv_p = Wv.rearrange("(kb p) o -> p kb o", p=128)
    Wo_p = Wo.rearrange("(kb p) o -> p kb o", p=128)
    outT_p = outT.rearrange("(ob p) t -> p ob t", p=128)

    with tile.TileContext(nc) as tc:
        with tc.tile_pool(name="persist", bufs=1) as persist, \
             tc.tile_pool(name="consts", bufs=1) as consts, \
             tc.tile_pool(name="stream", bufs=2) as stream, \
             tc.tile_pool(name="wqo", bufs=2) as wqo, \
             tc.tile_pool(name="exps", bufs=8) as exps, \
             tc.tile_pool(name="zrp", bufs=2) as zrp, \
             tc.tile_pool(name="dramp", bufs=1, space="DRAM") as dramp, \
             tc.tile_pool(name="ps512", bufs=2, space="PSUM") as ps512, \
             tc.tile_pool(name="ps1024", bufs=2, space="PSUM") as ps1024, \
             tc.tile_pool(name="psc", bufs=2, space="PSUM") as pscp:

            ones_f32 = consts.tile([1, max(sq, WN, 128)], F32)
            nc.vector.memset(ones_f32, 1.0)
            ones = consts.tile([1, max(sq, WN, 128)], F32R)
            nc.vector.tensor_copy(ones, ones_f32)
            ones_rep = consts.tile([128, HPH], F32)
            nc.vector.memset(ones_rep, 1.0)
            bq_sb = consts.tile([1, h], F32R, tag="bq")
            bk_sb = consts.tile([1, h], F32R, tag="bk")
            bv_sb = consts.tile([1, h], F32R, tag="bv")
            bo_sb = consts.tile([1, h], F32R, tag="bo")
            nc.sync.dma_start(out=bq_sb, in_=bq)
            nc.sync.dma_start(out=bk_sb, in_=bk)
            nc.sync.dma_start(out=bv_sb, in_=bv)
            nc.sync.dma_start(out=bo_sb, in_=bo)

            # normalized merged-head context, transposed: [128, MB, sq]
            ctxnT = persist.tile([128, MB, sq], F32R, tag="ctxnT")

            loop_cm = tc.For_i(0, reps, 1) if reps else contextlib.nullcontext()
            with loop_cm:
              for _srep in range(sreps):
                for hf in range(2):
                    hoff = hf * WH

                    # ---- resident weights for this half (1 DMA each;
                    # Wv deferred past the first kT tile so the first kpT
                    # matmul isn't queued behind 4MB of weight DMA) ----
                    Wk_sb = persist.tile([128, KB, WH], F32R, tag="wk")
                    Wv_sb = persist.tile([128, KB, WH], F32R, tag="wv")
                    nc.sync.dma_start(out=Wk_sb,
                                      in_=Wk_p[:, :, hoff:hoff + WH])

                    # ---- kp^T projection: [ho(128 x MH), kt] ----
                    kpT = persist.tile([128, MH, s], F32R, tag="kpT")
                    if ag:
                        agki = dramp.tile([128, MH, SL], F32R, tag="agki")
                        agko = dramp.tile([4, 128, MH, SL], F32R, tag="agko")
                    for n in range(SL // WN):
                        kt_t = stream.tile([128, KB, WN], F32R, tag="st2mb",
                                           name="kt_t")
                        nc.sync.dma_start(
                            out=kt_t, in_=kT_p[:, :, n * WN:(n + 1) * WN])
                        for m in range(MH):
                            ps = ps512.tile([128, WN], F32, tag="ps512")
                            for kb in range(KB):
                                nc.tensor.matmul(
                                    ps, Wk_sb[:, kb, m * 128:(m + 1) * 128],
                                    kt_t[:, kb, :], start=(kb == 0),
                                    stop=False)
                            nc.tensor.matmul(
                                ps,
                                bk_sb[0:1, hoff + m * 128:hoff + (m + 1) * 128],
                                ones[0:1, 0:WN], start=False, stop=True)
                            if ag:
                                kst = wqo.tile([128, WN], F32R, tag="kst")
                                nc.vector.tensor_copy(kst, ps)
                                nc.sync.dma_start(
                                    out=agki[:, m, n * WN:(n + 1) * WN],
                                    in_=kst)
                            else:
                                nc.vector.tensor_copy(
                                    kpT[:, m, n * WN:(n + 1) * WN], ps)

                    # ---- vp projection (token-major) + ones column ----
                    # vp[kt-part, ktb, head_local, 0:64] ; [.., 64] = 1.0
                    nc.sync.dma_start(out=Wv_sb,
                                      in_=Wv_p[:, :, hoff:hoff + WH])
                    vp = persist.tile([128, KTB, HPH, hd + 1], BF16, tag="vp")
                    if ag:
                        KTL = SL // 128
                        vps = persist.tile([128, KTL, HPH, hd + 1], BF16,
                                           tag="vps")
                        agvi = dramp.tile([128, KTL, HPH, hd + 1], BF16,
                                          tag="agvi")
                        agvo = dramp.tile([4, 128, KTL, HPH, hd + 1], BF16,
                                          tag="agvo")
                    else:
                        vps = vp
                    for n in range(SL // WN):
                        vt_t = stream.tile([128, KB, WN], F32R, tag="st2mb",
                                           name="vt_t")
                        nc.sync.dma_start(
                            out=vt_t, in_=vT_p[:, :, n * WN:(n + 1) * WN])
                        for st in range(TPW):
                            t = n * TPW + st
                            nc.vector.tensor_copy(vps[:, t, :, hd:hd + 1],
                                                  ones_rep)
                            ps = ps512.tile([128, WH], F32, tag="ps512")
                            for kb in range(KB):
                                nc.tensor.matmul(
                                    ps, vt_t[:, kb, st * 128:(st + 1) * 128],
                                    Wv_sb[:, kb, :], start=(kb == 0),
                                    stop=False)
                            nc.tensor.matmul(ps, ones[0:1, 0:128],
                                             bv_sb[0:1, hoff:hoff + WH],
                                             start=False, stop=True)
                            nc.vector.tensor_copy(
                                vps[:, t, :, 0:hd],
                                ps.rearrange("p (hh d) -> p hh d", d=hd))
                    if ag:
                        nc.sync.dma_start(out=agvi, in_=vps)
                        nc.gpsimd.collective_compute(
                            "AllGather", mybir.AluOpType.bypass,
                            ins=[agki.opt()], outs=[agko.opt()],
                            replica_groups=[[0, 1, 2, 3], [4, 5, 6, 7]])
                        nc.gpsimd.collective_compute(
                            "AllGather", mybir.AluOpType.bypass,
                            ins=[agvi.opt()], outs=[agvo.opt()],
                            replica_groups=[[0, 1, 2, 3], [4, 5, 6, 7]])

                    # ---- qp^T projection: [ho(128 x MH), qt] ----
                    qT_sb = stream.tile([128, KB, sq], F32R, tag="st2mb",
                                        name="qT_sb")
                    nc.sync.dma_start(out=qT_sb, in_=qT_p)
                    qpT = persist.tile([128, MH, sq], F32R, tag="qpT")
                    wq_t = stream.tile([128, KB, WH], F32R, tag="st2mb",
                                       name="wq_t")
                    nc.sync.dma_start(out=wq_t,
                                      in_=Wq_p[:, :, hoff:hoff + WH])
                    for m in range(MH):
                        ps = ps512.tile([128, sq], F32, tag="ps512")
                        for kb in range(KB):
                            nc.tensor.matmul(
                                ps, wq_t[:, kb, m * 128:(m + 1) * 128],
                                qT_sb[:, kb, :], start=(kb == 0), stop=False)
                        nc.tensor.matmul(
                            ps,
                            bq_sb[0:1, hoff + m * 128:hoff + (m + 1) * 128],
                            ones[0:1, 0:sq], start=False, stop=True)
                        nc.vector.tensor_copy(qpT[:, m, :], ps)

                    if ag:
                        nc.sync.dma_start(
                            out=kpT.rearrange("p m (r t) -> p m r t", r=4),
                            in_=agko.rearrange("r p m t -> p m r t"))
                        nc.sync.dma_start(
                            out=vp.rearrange("p (r k) h c -> p r k h c", r=4),
                            in_=agvo.rearrange("r p k h c -> p r k h c"))

                    if upto < 2:
                        # consume proj outputs so DCE keeps them (timing mode)
                        nc.sync.dma_start(out=outT_p[:, 0, :],
                                          in_=kpT[:, 0, 0:sq].bitcast(F32))
                        nc.sync.dma_start(out=outT_p[:, 1, :],
                                          in_=qpT[:, 0, :].bitcast(F32))
                        nc.gpsimd.dma_start(
                            out=outT_p[:, 2, 0:65],
                            in_=vp[:, hf, 0, :])
                        continue
                    # ---- attention, processed in head pairs ----
                    # scores 2 ktb per 2-bank psum tile -> one exp per
                    # [128, 2*sq]; ctx MMs software-pipelined one chunk
                    # behind so ACT streams without PE ping-pong.
                    CH = min(4 if v2 else 2, KTB)  # ktb per chunk
                    n_ps_bufs = 1 if v2 else 2
                    # Two pairs processed CONCURRENTLY: each pair's
                    # scores->exp->ctx chain has ~1us cross-engine latency
                    # bubbles per chunk; the sibling pair's independent
                    # chunks fill them.  The odd pair's ctx accumulators
                    # live in the ps512 pool (idle during attention), so
                    # PSUM stays exactly at 8 banks.
                    PG = 2 if HPH // 2 >= 2 else 1
                    for prp in range(0, HPH // 2, PG):
                        prs = list(range(prp, min(prp + PG, HPH // 2)))
                        pscs = []
                        for pi in range(len(prs)):
                            pool = pscp if pi == 0 else ps512
                            tag = "psc" if pi == 0 else "ps512"
                            pscs.append([
                                pool.tile([hd + 1, sq], F32, tag=tag,
                                          name=f"psc{pi}_{j}")
                                for j in range(2)])
                        prevs = [None for _ in prs]
                        for cc in range(KTB // CH):
                            for pi, pr in enumerate(prs):
                                m = pr
                                psc = pscs[pi]
                                p1s = [ps1024.tile([128, CH, sq], F32,
                                                   tag="ps1024",
                                                   bufs=n_ps_bufs,
                                                   name=f"p1_{pi}_{j}")
                                       for j in range(2)]
                                for i in range(CH):
                                    ktb = cc * CH + i
                                    for j, roff in enumerate((0, 64)):
                                        nc.tensor.matmul(
                                            p1s[j][:, i, :],
                                            kpT[roff:roff + 64, m,
                                                ktb * 128:(ktb + 1) * 128],
                                            qpT[roff:roff + 64, m, :],
                                            start=True, stop=True)
                                ets = []
                                for j in range(2):
                                    if exp_sbuf and (exp_sbuf == "all"
                                                     or (cc + j) % 2 == 0):
                                        sc = exps.tile([128, CH, sq], F32,
                                                       tag="sc_t", bufs=2,
                                                       name=f"sc_{pi}_{j}")
                                        nc.vector.tensor_copy(sc, p1s[j])
                                    else:
                                        sc = p1s[j]
                                    et = exps.tile([128, CH, sq], BF16,
                                                   tag="exp_t",
                                                   bufs=4 if v2 else 8,
                                                   name=f"et_{pi}_{j}")
                                    nc.scalar.activation(
                                        out=et, in_=sc,
                                        func=mybir.ActivationFunctionType.Exp)
                                    ets.append(et)
                                if prevs[pi] is not None:
                                    pcc, pets = prevs[pi]
                                    for j in range(2):
                                        for i in range(CH):
                                            ktb = pcc * CH + i
                                            nc.tensor.matmul(
                                                psc[j],
                                                vp[:, ktb, 2 * pr + j, :],
                                                pets[j][:, i, :],
                                                start=(ktb == 0),
                                                stop=(ktb == KTB - 1))
                                prevs[pi] = (cc, ets)
                        for pi, pr in enumerate(prs):
                            m = pr
                            psc = pscs[pi]
                            pcc, pets = prevs[pi]
                            for j in range(2):
                                for i in range(CH):
                                    ktb = pcc * CH + i
                                    nc.tensor.matmul(
                                        psc[j], vp[:, ktb, 2 * pr + j, :],
                                        pets[j][:, i, :],
                                        start=(ktb == 0),
                                        stop=(ktb == KTB - 1))
                            # normalize: ctxn = ctx * (1/Z), Z-broadcast on
                            # the otherwise-idle gpsimd engine
                            for j, roff in enumerate((0, 64)):
                                zr = zrp.tile([1, sq], F32, tag="zr", bufs=2)
                                with nc.allow_low_precision(
                                        reason="1/Z of softmax; DVE mul"):
                                    nc.vector.reciprocal(
                                        zr, psc[j][hd:hd + 1, :])
                                zb = zrp.tile([64, sq], F32, tag="zb",
                                              bufs=2)
                                nc.gpsimd.partition_broadcast(zb, zr)
                                nc.vector.tensor_mul(
                                    ctxnT[roff:roff + 64, hf * MH + m, :],
                                    psc[j][0:hd, :], zb)

                if upto < 3:
                    if upto == 2:
                        nc.sync.dma_start(out=outT_p[:, 0, :],
                                          in_=ctxnT[:, 0, :].bitcast(F32))
                    continue_marker = True
                # ---- output projection: outT[o, t] ----
                for ob in range(MB if upto >= 3 else 0):
                    wo_t = wqo.tile([128, KB, 128], F32R, tag="wo_t")
                    nc.sync.dma_start(
                        out=wo_t, in_=Wo_p[:, :, ob * 128:(ob + 1) * 128])
                    po = ps512.tile([128, sq], F32, tag="ps512")
                    for mb in range(MB):
                        nc.tensor.matmul(po, wo_t[:, mb, :], ctxnT[:, mb, :],
                                         start=(mb == 0), stop=False)
                    nc.tensor.matmul(po,
                                     bo_sb[0:1, ob * 128:(ob + 1) * 128],
                                     ones[0:1, 0:sq], start=False, stop=True)
                    ot = wqo.tile([128, sq], F32, tag="ot")
                    nc.vector.tensor_copy(ot, po)
                    nc.sync.dma_start(out=outT_p[:, ob, :], in_=ot)

    nc.compile()
    return nc


def shard_inputs(q, k, v, Wq, bq, Wk, bk, Wv, bv, Wo, bo,
                 s=S, h=H, sq=SQ, n_cores=N_CORES, cpg=CPG, ag=False):
    """Host-side sharding: per-core input dicts (numpy, fp32, contiguous)."""
    scale = np.float32(1.0 / np.sqrt(HD))
    c32 = lambda a: np.ascontiguousarray(a, dtype=np.float32)
    Wq_s, bq_s = c32(Wq) * scale, c32(bq).reshape(1, h) * scale
    Wk_c, bk_c = c32(Wk), c32(bk).reshape(1, h)
    Wv_c, bv_c = c32(Wv), c32(bv).reshape(1, h)
    Wo_c, bo_c = c32(Wo), c32(bo).reshape(1, h)
    kT_b = [c32(k[b].T) for b in range(q.shape[0])]
    vT_b = [c32(v[b].T) for b in range(q.shape[0])]
    in_maps = []
    for c in range(n_cores):
        b, r0 = c // cpg, (c % cpg) * sq
        rr = c % cpg
        ksl = kT_b[b] if not ag else c32(kT_b[b][:, rr * (s // 4):(rr + 1) * (s // 4)])
        vsl = vT_b[b] if not ag else c32(vT_b[b][:, rr * (s // 4):(rr + 1) * (s // 4)])
        in_maps.append({
            "qT": c32(q[b, r0:r0 + sq, :].T),
            "kT": ksl, "vT": vsl,
            "Wq": Wq_s, "bq": bq_s, "Wk": Wk_c, "bk": bk_c,
            "Wv": Wv_c, "bv": bv_c, "Wo": Wo_c, "bo": bo_c,
        })
    return in_maps


_NC_CACHE = {}


AG = False  # set True to use the 4-rank AllGather variant


def get_nc():
    if "nc" not in _NC_CACHE:
        _NC_CACHE["nc"] = build_nc(ag=AG)
    return _NC_CACHE["nc"]


def kernel(q, k, v, Wq, bq, Wk, bk, Wv, bv, Wo, bo):
    q, k, v = np.asarray(q), np.asarray(k), np.asarray(v)
    in_maps = shard_inputs(q, k, v, Wq, bq, Wk, bk, Wv, bv, Wo, bo, ag=AG)
    nc = get_nc()
    res = run_bass_kernel_spmd(nc, in_maps, core_ids=list(range(N_CORES)))
    out = np.empty((B, S, H), dtype=np.float32)
    for c in range(N_CORES):
        b, r0 = c // CPG, (c % CPG) * SQ
        out[b, r0:r0 + SQ, :] = res.results[c]["outT"].T
    return out



# revision 9
# speedup vs baseline: 1.2618x; 1.2618x over previous
"""Trainium2 Bass kernel for DeTrAttention (dense transformer MHA block).

Full op: out = softmax((q@Wq+bq)(k@Wk+bk)^T / sqrt(64)) (v@Wv+bv) @ Wo + bo
Shapes: q,k,v [B=2, S=2048, H=1024], NH=16 heads, HD=64.

Sharding (8 cores): data-parallel over batch (2 groups of 4 cores); within a
group core r owns query rows [512r, 512r+512) end-to-end AND computes the
K/V projections only for ITS 512 tokens; a single merged 4-rank AllGather
(byte-packed kp bf16 + vp fp8, 1.06MB -> 4.2MB via DRAM staging) then gives
every core the full-sequence kp/vp.  This removes the 4x-redundant K/V
projection compute of the all-local variant (~220k PE cycles/core).

Precision/engine strategy:
  - host passes q^T,k^T,v^T slices and Wq,Wk,Wv in bf16 (Wq,bq pre-scaled by
    1/sqrt(64)); projections accumulate in fp32 PSUM -> ~0.1% err
  - scores matmuls bf16 (kpT x qpT, contraction=head dim 64)
  - exp on the scalar engine, bf16 probabilities out.  fp8 probs/values were
    tested and REJECTED: with near-uniform attention (scores ~N(0,1), no
    sharp peaks) the context is a mean over ~750 effective keys (magnitude
    ~0.04), so fp8's 6% per-element noise does NOT average away relative to
    the signal -- measured 2.1e-2 rel err from vp-fp8 alone, 2.9e-2 with
    fp8 probs (budget 2e-2).  bf16 keeps it at ~2e-3.
  - ctx matmuls bf16; an appended ones-column in vp yields the denominator
    Z in row 64; normalization via reciprocal + K=1 gpsimd
    partition-broadcast + DVE multiply
  - biases are folded into the PSUM->SBUF copies as DVE tensor_scalar_add
    with per-partition [128,1] scalars (kills the rank-1 bias matmuls);
    only the token-major vp projection keeps a ones-vector bias matmul
  - output projection in f32r (full precision on the final matmul)
Per-core roofline: ~300k PE cycles (~125us at 2.4GHz), ~127us scalar-engine
exp, ~30MB DMA (~85us); the AllGather (~60-100us latency) overlaps the
q-projection and, across loop iterations, the previous attention phase.
"""

import numpy as np

import concourse.bass as bass
import concourse.tile as tile
from concourse import bacc, mybir
from concourse.bass_utils import run_bass_kernel_spmd

F32 = mybir.dt.float32
F32R = mybir.dt.float32r
BF16 = mybir.dt.bfloat16
FP8 = mybir.dt.float8e4
U8 = mybir.dt.uint8
DR = mybir.MatmulPerfMode.DoubleRow

B, S, H, NH = 2, 2048, 1024, 16
HD = H // NH  # 64
N_CORES = 8
CPG = 4            # cores per batch group
SQ = S // CPG      # query rows per core (512)
SL = S // CPG      # local K/V tokens per core (512)
KB = H // 128      # contraction 128-blocks (8)
MB = H // 128      # output-feature 128-blocks (8)
KTB = S // 128     # key-token 128-blocks (16)
KTL = SL // 128    # local key-token blocks (4)
EBIAS = -2.0       # exp(s + EBIAS): fp8 overflow guard, cancels in softmax

KPW = MB * SQ * 2              # kp bytes/partition in the gather payload
VPW = KTL * NH * (HD + 1) * 2  # vp bytes/partition (bf16)
AGW = KPW + VPW


def build_nc(sreps=1, upto=3, sim=False):
    """Per-core Bass program (SPMD, identical on all 8 cores).

    sreps > 1 statically unrolls the body for steady-state timing
    (collectives cannot live inside hardware control flow).
    sim=True replaces the AllGather with 4 local DMA broadcasts (same
    DRAM traffic shape) so single-core TimelineSim can model the schedule.
    """
    nc = bacc.Bacc("TRN2", target_bir_lowering=False, debug=False,
                   num_devices=8)

    qT = nc.dram_tensor("qT", [H, SQ], BF16, kind="ExternalInput").ap()
    kT = nc.dram_tensor("kT", [H, SL], BF16, kind="ExternalInput").ap()
    vT = nc.dram_tensor("vT", [H, SL], BF16, kind="ExternalInput").ap()
    Wq = nc.dram_tensor("Wq", [H, H], BF16, kind="ExternalInput").ap()
    Wk = nc.dram_tensor("Wk", [H, H], BF16, kind="ExternalInput").ap()
    Wv = nc.dram_tensor("Wv", [H, H], BF16, kind="ExternalInput").ap()
    Wo = nc.dram_tensor("Wo", [H, H], F32R, kind="ExternalInput").ap()
    bqT = nc.dram_tensor("bqT", [128, MB], F32, kind="ExternalInput").ap()
    bkT = nc.dram_tensor("bkT", [128, MB], F32, kind="ExternalInput").ap()
    boT = nc.dram_tensor("boT", [128, MB], F32, kind="ExternalInput").ap()
    bv = nc.dram_tensor("bv", [1, H], F32R, kind="ExternalInput").ap()
    outT = nc.dram_tensor("outT", [H, SQ], F32, kind="ExternalOutput").ap()

    # partition-major views so whole tensors move in one DMA
    qT_p = qT.rearrange("(kb p) t -> p kb t", p=128)
    kT_p = kT.rearrange("(kb p) t -> p kb t", p=128)
    vT_p = vT.rearrange("(kb p) t -> p kb t", p=128)
    Wq_p = Wq.rearrange("(kb p) o -> p kb o", p=128)
    Wk_p = Wk.rearrange("(kb p) o -> p kb o", p=128)
    Wv_p = Wv.rearrange("(kb p) o -> p kb o", p=128)
    Wo_p = Wo.rearrange("(kb p) o -> p kb o", p=128)
    outT_p = outT.rearrange("(ob p) t -> p ob t", p=128)

    GROUPS = [[0, 1, 2, 3], [4, 5, 6, 7]]

    with tile.TileContext(nc) as tc:
        with tc.tile_pool(name="consts", bufs=1) as consts, \
             tc.tile_pool(name="persist", bufs=1) as persist, \
             tc.tile_pool(name="stream", bufs=2) as stream, \
             tc.tile_pool(name="wq", bufs=2) as wqp, \
             tc.tile_pool(name="wo", bufs=2) as wop, \
             tc.tile_pool(name="exps", bufs=8) as exps, \
             tc.tile_pool(name="zrp", bufs=2) as zrp, \
             tc.tile_pool(name="otp", bufs=2) as otp, \
             tc.tile_pool(name="dramp", bufs=2, space="DRAM") as dramp, \
             tc.tile_pool(name="ps2b", bufs=2, space="PSUM") as ps2b, \
             tc.tile_pool(name="psc", bufs=4, space="PSUM") as pscp:

            ones_f = consts.tile([1, 128], F32)
            nc.vector.memset(ones_f, 1.0)
            ones = consts.tile([1, 128], F32R)
            nc.vector.tensor_copy(ones, ones_f)
            ones16 = consts.tile([128, NH], F32)
            nc.vector.memset(ones16, 1.0)
            ebias = consts.tile([128, 1], F32)
            nc.vector.memset(ebias, EBIAS)
            bq_sb = consts.tile([128, MB], F32, tag="bq")
            bk_sb = consts.tile([128, MB], F32, tag="bk")
            bo_sb = consts.tile([128, MB], F32, tag="bo")
            bv_sb = consts.tile([1, H], F32R, tag="bv")
            nc.sync.dma_start(out=bq_sb, in_=bqT)
            nc.sync.dma_start(out=bk_sb, in_=bkT)
            nc.sync.dma_start(out=bo_sb, in_=boT)
            nc.sync.dma_start(out=bv_sb, in_=bv)

            # persistent per-iteration state
            kpT = persist.tile([128, MB, S], BF16, tag="kpT")
            vp = persist.tile([128, KTB, NH, HD + 1], BF16, tag="vp")
            qpT = persist.tile([128, MB, SQ], BF16, tag="qpT")
            ctxnT = persist.tile([128, MB, SQ], F32R, tag="ctxnT")
            kpl = persist.tile([128, MB, SL], BF16, tag="kpl")
            vpl = persist.tile([128, KTL, NH, HD + 1], BF16, tag="vpl")

            for _srep in range(sreps):
                # ---- local K projection: kpl[ho, t] (bias fused) ----
                kt_t = stream.tile([128, KB, SL], BF16, tag="in3",
                                   name="kt_t")
                nc.sync.dma_start(out=kt_t, in_=kT_p)
                for wh in range(2):
                    wk_t = wqp.tile([128, KB, H // 2], BF16, tag="w")
                    nc.sync.dma_start(
                        out=wk_t, in_=Wk_p[:, :, wh * 512:(wh + 1) * 512])
                    for m in range(4):
                        mb = wh * 4 + m
                        ps = pscp.tile([128, SL], F32, tag="psc")
                        for kb in range(KB):
                            nc.tensor.matmul(
                                ps, wk_t[:, kb, m * 128:(m + 1) * 128],
                                kt_t[:, kb, :], start=(kb == 0),
                                stop=(kb == KB - 1))
                        nc.vector.tensor_scalar_add(
                            kpl[:, mb, :], ps, bk_sb[:, mb:mb + 1])

                # ---- local V projection (token-major) + ones column ----
                vt_t = stream.tile([128, KB, SL], BF16, tag="in3",
                                   name="vt_t")
                nc.sync.dma_start(out=vt_t, in_=vT_p)
                wv_ts = []
                for wh in range(2):
                    wv_t = wqp.tile([128, KB, H // 2], BF16, tag="w")
                    nc.sync.dma_start(
                        out=wv_t, in_=Wv_p[:, :, wh * 512:(wh + 1) * 512])
                    wv_ts.append(wv_t)
                for st in range(KTL):
                    ps = ps2b.tile([128, H], F32, tag="ps2b")
                    for wh in range(2):
                        sl = slice(wh * 512, (wh + 1) * 512)
                        for kb in range(KB):
                            nc.tensor.matmul(
                                ps[:, sl],
                                vt_t[:, kb, st * 128:(st + 1) * 128],
                                wv_ts[wh][:, kb, :], start=(kb == 0),
                                stop=False)
                        nc.tensor.matmul(ps[:, sl], ones, bv_sb[0:1, sl],
                                         start=False, stop=True)
                    nc.vector.tensor_copy(
                        vpl[:, st, :, 0:HD],
                        ps.rearrange("p (hh d) -> p hh d", d=HD))
                    nc.vector.tensor_copy(
                        vpl[:, st, :, HD:HD + 1],
                        ones16.rearrange("p (hh o) -> p hh o", o=1))

                # ---- merged AllGather of (kpl, vpl), byte-packed ----
                agi = dramp.tile([128, AGW], U8, tag="agi")
                ago = dramp.tile([CPG, 128, AGW], U8, tag="ago")
                nc.sync.dma_start(
                    out=agi[:, 0:KPW],
                    in_=kpl.rearrange("p m t -> p (m t)").bitcast(U8))
                nc.sync.dma_start(
                    out=agi[:, KPW:AGW],
                    in_=vpl.rearrange("p k h c -> p (k h c)").bitcast(U8))
                if sim:
                    for r in range(CPG):
                        nc.sync.dma_start(out=ago[r], in_=agi)
                else:
                    nc.gpsimd.collective_compute(
                        "AllGather", mybir.AluOpType.bypass,
                        ins=[agi.opt()], outs=[ago.opt()],
                        replica_groups=GROUPS)

                # ---- q projection (both halves; overlaps the AllGather) ----
                qt_t = stream.tile([128, KB, SQ], BF16, tag="in3",
                                   name="qt_t")
                nc.sync.dma_start(out=qt_t, in_=qT_p)
                for wh in range(2):
                    wq_t = wqp.tile([128, KB, H // 2], BF16, tag="w")
                    nc.sync.dma_start(
                        out=wq_t, in_=Wq_p[:, :, wh * 512:(wh + 1) * 512])
                    for m in range(4):
                        mb = wh * 4 + m
                        ps = pscp.tile([128, SQ], F32, tag="psc")
                        for kb in range(KB):
                            nc.tensor.matmul(
                                ps, wq_t[:, kb, m * 128:(m + 1) * 128],
                                qt_t[:, kb, :], start=(kb == 0),
                                stop=(kb == KB - 1))
                        nc.vector.tensor_scalar_add(
                            qpT[:, mb, :], ps, bq_sb[:, mb:mb + 1])

                # ---- scatter gathered kp/vp into attention layout ----
                for r in range(CPG):
                    nc.sync.dma_start(
                        out=kpT[:, :, r * SL:(r + 1) * SL],
                        in_=ago[r, :, 0:KPW].bitcast(BF16).rearrange(
                            "p (m t) -> p m t", t=SL))
                    nc.sync.dma_start(
                        out=vp[:, r * KTL:(r + 1) * KTL],
                        in_=ago[r, :, KPW:AGW].bitcast(BF16).rearrange(
                            "p (k h c) -> p k h c", h=NH, c=HD + 1))

                if upto < 2:
                    nc.sync.dma_start(out=outT_p[:, 0, :],
                                      in_=kpT[:, 0, 0:SQ].bitcast(F32))
                    nc.sync.dma_start(out=outT_p[:, 1, :],
                                      in_=qpT[:, 0, :].bitcast(F32))
                    nc.gpsimd.dma_start(out=outT_p[:, 2, 0:65],
                                        in_=vp[:, 0, 0, :])
                    continue

                # ---- attention: 2 head-pairs in flight, 2 ktb per chunk,
                # ctx via fp8 DoubleRow with Z in row 64 ----
                for hf in range(2):
                    for prp in range(0, 4, 2):
                        prs = [prp, prp + 1]
                        pscs = [[pscp.tile([128, SQ], F32, tag="psc",
                                           name=f"psc{pi}_{j}")
                                 for j in range(2)] for pi in range(2)]
                        for cc in range(KTB // 2):
                            for pi, pr in enumerate(prs):
                                mb = hf * 4 + pr
                                for j in range(2):
                                    p1 = ps2b.tile([128, 2, SQ], F32,
                                                   tag="ps2b",
                                                   name=f"p1_{pi}_{j}")
                                    for i in range(2):
                                        ktb = cc * 2 + i
                                        nc.tensor.matmul(
                                            p1[:, i, :],
                                            kpT[64 * j:64 * j + 64, mb,
                                                ktb * 128:(ktb + 1) * 128],
                                            qpT[64 * j:64 * j + 64, mb, :],
                                            start=True, stop=True)
                                    et = exps.tile([128, 2, SQ], BF16,
                                                   tag="et",
                                                   name=f"et_{pi}_{j}")
                                    nc.scalar.activation(
                                        out=et, in_=p1,
                                        func=mybir.ActivationFunctionType.Exp,
                                        bias=ebias[:, 0:1])
                                    h = 2 * mb + j
                                    for i in range(2):
                                        ktb = cc * 2 + i
                                        nc.tensor.matmul(
                                            pscs[pi][j][0:HD + 1, :],
                                            vp[:, ktb, h, :],
                                            et[:, i, :], start=(ktb == 0),
                                            stop=(ktb == KTB - 1))
                        for pi, pr in enumerate(prs):
                            mb = hf * 4 + pr
                            for j in range(2):
                                psc = pscs[pi][j]
                                zr = zrp.tile([1, SQ], F32, tag="zr")
                                with nc.allow_low_precision(
                                        reason="softmax 1/Z; DVE mul"):
                                    nc.vector.reciprocal(
                                        zr, psc[HD:HD + 1, :])
                                zb = zrp.tile([64, SQ], F32, tag="zb")
                                nc.gpsimd.partition_broadcast(zb, zr)
                                nc.vector.tensor_mul(
                                    ctxnT[64 * j:64 * j + 64, mb, :],
                                    psc[0:HD, :], zb)

                if upto < 3:
                    nc.sync.dma_start(out=outT_p[:, 0, :],
                                      in_=ctxnT[:, 0, :].bitcast(F32))
                    continue

                # ---- output projection (f32r, bias fused) ----
                for ob in range(MB):
                    wo_t = wop.tile([128, KB, 128], F32R, tag="wo_t")
                    nc.sync.dma_start(
                        out=wo_t, in_=Wo_p[:, :, ob * 128:(ob + 1) * 128])
                    po = ps2b.tile([128, H], F32, tag="ps2b")
                    for mb in range(MB):
                        nc.tensor.matmul(po[:, 0:SQ], wo_t[:, mb, :],
                                         ctxnT[:, mb, :],
                                         start=(mb == 0), stop=(mb == MB - 1))
                    ot = otp.tile([128, SQ], F32, tag="ot")
                    nc.vector.tensor_scalar_add(ot, po[:, 0:SQ],
                                                bo_sb[:, ob:ob + 1])
                    nc.sync.dma_start(out=outT_p[:, ob, :], in_=ot)

    nc.compile()
    return nc


def shard_inputs(q, k, v, Wq, bq, Wk, bk, Wv, bv, Wo, bo):
    """Host-side sharding: per-core input dicts."""
    import ml_dtypes
    bf16 = ml_dtypes.bfloat16
    scale = np.float32(1.0 / np.sqrt(HD))
    c32 = lambda a: np.ascontiguousarray(a, dtype=np.float32)
    cbf = lambda a: np.ascontiguousarray(np.asarray(a, dtype=np.float32),
                                         dtype=bf16)
    Wq_b = cbf(c32(Wq) * scale)
    Wk_b, Wv_b = cbf(Wk), cbf(Wv)
    Wo_c = c32(Wo)
    bqT = np.ascontiguousarray((c32(bq) * scale).reshape(MB, 128).T)
    bkT = np.ascontiguousarray(c32(bk).reshape(MB, 128).T)
    boT = np.ascontiguousarray(c32(bo).reshape(MB, 128).T)
    bv_c = c32(bv).reshape(1, H)
    in_maps = []
    for c in range(N_CORES):
        b, r = c // CPG, c % CPG
        sl = slice(r * SQ, (r + 1) * SQ)
        in_maps.append({
            "qT": cbf(q[b, sl, :].T), "kT": cbf(k[b, sl, :].T),
            "vT": cbf(v[b, sl, :].T),
            "Wq": Wq_b, "Wk": Wk_b, "Wv": Wv_b, "Wo": Wo_c,
            "bqT": bqT, "bkT": bkT, "boT": boT, "bv": bv_c,
        })
    return in_maps


_NC_CACHE = {}


def get_nc():
    if "nc" not in _NC_CACHE:
        _NC_CACHE["nc"] = build_nc()
    return _NC_CACHE["nc"]


def kernel(q, k, v, Wq, bq, Wk, bk, Wv, bv, Wo, bo):
    q, k, v = np.asarray(q), np.asarray(k), np.asarray(v)
    in_maps = shard_inputs(q, k, v, Wq, bq, Wk, bk, Wv, bv, Wo, bo)
    nc = get_nc()
    res = run_bass_kernel_spmd(nc, in_maps, core_ids=list(range(N_CORES)))
    out = np.empty((B, S, H), dtype=np.float32)
    for c in range(N_CORES):
        b, r0 = c // CPG, (c % CPG) * SQ
        out[b, r0:r0 + SQ, :] = res.results[c]["outT"].T
    return out


# revision 14
# speedup vs baseline: 2.4310x; 1.9267x over previous
"""Trainium2 Bass kernel for DeTrAttention (dense transformer MHA block).

Full op: out = softmax((q@Wq+bq)(k@Wk+bk)^T / sqrt(64)) (v@Wv+bv) @ Wo + bo
Shapes: q,k,v [B=2, S=2048, H=1024], NH=16 heads, HD=64.

Sharding (8 cores): data-parallel over batch (2 groups of 4 cores); within a
group core r owns query rows [512r, 512r+512) end-to-end AND computes the
K/V projections only for ITS 512 tokens; a single merged byte-packed 4-rank
AllGather (1.05MB -> 4.2MB via DRAM staging) then gives every core the
full-sequence kp/vp.  This removes the 4x-redundant K/V projection compute
(~220k PE cycles/core) of the all-local variant.

Schedule: the Tensor engine only reaches 2.4GHz after ~3us of gap-free
execution (1.2GHz otherwise), and the attention phase alone is exp-paced
(scalar engine), leaving ~1us PE gaps per chunk.  The body is therefore
software-pipelined: iteration i+1's K/V/Q projection matmul groups (and its
AllGather launch) are EMITTED between iteration i's attention chunks, so
the PE queue never drains and the collective latency hides behind a full
attention phase.  kpT/vp/qpT are double-buffered (A/B by iteration parity)
to break the scatter-vs-attention WAR that otherwise serializes the
collective.

Precision: inputs/weights bf16 (Wq,bq pre-scaled by 1/sqrt(64)), fp32 PSUM
accumulation, bf16 scores/probs/values/output-proj (~5e-3 rel err overall).
fp8 attention (DoubleRow) was tested and REJECTED: with near-uniform
attention the context is a mean over ~750 keys (magnitude ~0.04), so fp8's
6% per-element noise does not average away -- measured 2.1e-2 from vp-fp8
alone.  exp(s - 2) guards nothing here in bf16 but costs nothing and keeps
probs in a friendly range; the -2 cancels exactly in the softmax
normalization (ones-column in vp accumulates Z in PSUM row 64).
Biases fold into the PSUM->SBUF copies as per-partition tensor_scalar_adds
(the token-major vp bias arrives pre-broadcast from the host).
"""

import numpy as np

import concourse.bass as bass
import concourse.tile as tile
from concourse import bacc, mybir
from concourse.bass_utils import run_bass_kernel_spmd

F32 = mybir.dt.float32
F32R = mybir.dt.float32r
BF16 = mybir.dt.bfloat16
U8 = mybir.dt.uint8

B, S, H, NH = 2, 2048, 1024, 16
HD = H // NH  # 64
N_CORES = 8
CPG = 4            # cores per batch group
SQ = S // CPG      # query rows per core (512)
SL = S // CPG      # local K/V tokens per core (512)
KB = H // 128      # contraction 128-blocks (8)
MB = H // 128      # output-feature 128-blocks (8)
KTB = S // 128     # key-token 128-blocks (16)
KTL = SL // 128    # local key-token blocks (4)
EBIAS = -2.0       # exp(s + EBIAS); cancels in softmax

KPW = MB * SL * 2              # kp bytes/partition in the gather payload
VW = NH * (HD + 1) * 2         # vp bytes/partition per token-block (2080)
VPW = KTL * VW
AGW = KPW + VPW
GROUPS = [[0, 1, 2, 3], [4, 5, 6, 7]]


def build_nc(sreps=1, upto=3, sim=False):
    """Per-core Bass program (SPMD, identical on all 8 cores).

    sreps > 1 statically unrolls the body for steady-state timing
    (collectives cannot live inside hardware control flow).
    sim=True replaces the AllGather with 4 local DMA broadcasts (same
    DRAM traffic shape) so single-core TimelineSim can model the schedule.
    """
    nc = bacc.Bacc("TRN2", target_bir_lowering=False, debug=False,
                   num_devices=8)

    qT = nc.dram_tensor("qT", [H, SQ], BF16, kind="ExternalInput").ap()
    kT = nc.dram_tensor("kT", [H, SL], BF16, kind="ExternalInput").ap()
    vT = nc.dram_tensor("vT", [H, SL], BF16, kind="ExternalInput").ap()
    Wq = nc.dram_tensor("Wq", [H, H], BF16, kind="ExternalInput").ap()
    Wk = nc.dram_tensor("Wk", [H, H], BF16, kind="ExternalInput").ap()
    Wv = nc.dram_tensor("Wv", [H, H], BF16, kind="ExternalInput").ap()
    Wo = nc.dram_tensor("Wo", [H, H], BF16, kind="ExternalInput").ap()
    bqT = nc.dram_tensor("bqT", [128, MB], F32, kind="ExternalInput").ap()
    bkT = nc.dram_tensor("bkT", [128, MB], F32, kind="ExternalInput").ap()
    boT = nc.dram_tensor("boT", [128, MB], F32, kind="ExternalInput").ap()
    # bv pre-broadcast to all partitions, with a trailing 1.0 column per head
    bvp = nc.dram_tensor("bvp", [128, NH, HD + 1], BF16,
                         kind="ExternalInput").ap()
    outT = nc.dram_tensor("outT", [H, SQ], BF16, kind="ExternalOutput").ap()

    # partition-major views so whole tensors move in one DMA
    qT_p = qT.rearrange("(kb p) t -> p kb t", p=128)
    kT_p = kT.rearrange("(kb p) t -> p kb t", p=128)
    vT_p = vT.rearrange("(kb p) t -> p kb t", p=128)
    Wq_p = Wq.rearrange("(kb p) o -> p kb o", p=128)
    Wk_p = Wk.rearrange("(kb p) o -> p kb o", p=128)
    Wv_p = Wv.rearrange("(kb p) o -> p kb o", p=128)
    Wo_p = Wo.rearrange("(kb p) o -> p kb o", p=128)
    outT_p = outT.rearrange("(ob p) t -> p ob t", p=128)

    with tile.TileContext(nc) as tc:
        with tc.tile_pool(name="consts", bufs=1) as consts, \
             tc.tile_pool(name="persist", bufs=1) as persist, \
             tc.tile_pool(name="stream", bufs=2) as stream, \
             tc.tile_pool(name="wq", bufs=2) as wqp, \
             tc.tile_pool(name="wo", bufs=2) as wop, \
             tc.tile_pool(name="exps", bufs=4) as exps, \
             tc.tile_pool(name="zrp", bufs=1) as zrp, \
             tc.tile_pool(name="stg", bufs=2) as stgp, \
             tc.tile_pool(name="dramp", bufs=2, space="DRAM") as dramp, \
             tc.tile_pool(name="ps2b", bufs=2, space="PSUM") as ps2b, \
             tc.tile_pool(name="psa", bufs=2, space="PSUM") as psA, \
             tc.tile_pool(name="psacc", bufs=2, space="PSUM") as psacc:

            ebias = consts.tile([128, 1], F32)
            nc.vector.memset(ebias, EBIAS)
            bq_sb = consts.tile([128, MB], F32, tag="bq")
            bk_sb = consts.tile([128, MB], F32, tag="bk")
            bo_sb = consts.tile([128, MB], F32, tag="bo")
            bvp_sb = consts.tile([128, NH, HD + 1], BF16, tag="bvp")
            nc.sync.dma_start(out=bq_sb, in_=bqT)
            nc.sync.dma_start(out=bk_sb, in_=bkT)
            nc.sync.dma_start(out=bo_sb, in_=boT)
            nc.sync.dma_start(out=bvp_sb, in_=bvp)

            # double-buffered per-iteration state (parity = iteration % 2)
            kpTs = [persist.tile([128, MB, S], BF16, tag=f"kpT{x}",
                                 name=f"kpT{x}") for x in "AB"]
            vps = [persist.tile([128, KTB, NH, HD + 1], BF16, tag=f"vp{x}",
                                name=f"vp{x}") for x in "AB"]
            qpTs = [persist.tile([128, MB, SQ], BF16, tag=f"qpT{x}",
                                 name=f"qpT{x}") for x in "AB"]
            ctxnT = persist.tile([128, MB, SQ], BF16, tag="ctxnT")

            def emit_iter_inputs(it):
                """Emitter closures for iteration `it`'s input pipeline:
                K/V projections staged to DRAM, AllGather launch, Q
                projection.  Each closure is a ~8-matmul unit suitable for
                interleaving into the previous iteration's attention."""
                par = it % 2
                st8 = {}
                ems = []

                def e_kbegin():
                    st8["agi"] = dramp.tile([128, AGW], U8, tag="agi",
                                            name="agi")
                    st8["ago"] = dramp.tile([CPG, 128, AGW], U8, tag="ago",
                                            name="ago")
                    kt_t = stream.tile([128, KB, SL], BF16, tag="in3",
                                       name="kt_t")
                    nc.sync.dma_start(out=kt_t, in_=kT_p)
                    st8["kt"] = kt_t
                    for wh in range(2):
                        w = wqp.tile([128, KB, H // 2], BF16, tag="w",
                                     name=f"wk{wh}")
                        nc.sync.dma_start(
                            out=w, in_=Wk_p[:, :, wh * 512:(wh + 1) * 512])
                        st8[f"wk{wh}"] = w
                ems.append(e_kbegin)

                def e_kp(mb):
                    wh, m = mb // 4, mb % 4
                    ps = psA.tile([128, SL], F32, tag="psa", name=f"kp{mb}")
                    for kb in range(KB):
                        nc.tensor.matmul(
                            ps, st8[f"wk{wh}"][:, kb, m * 128:(m + 1) * 128],
                            st8["kt"][:, kb, :], start=(kb == 0),
                            stop=(kb == KB - 1))
                    kst = stgp.tile([128, SL], BF16, tag="kst", name="kst")
                    nc.vector.tensor_scalar_add(kst, ps, bk_sb[:, mb:mb + 1])
                    nc.sync.dma_start(
                        out=st8["agi"][:, mb * SL * 2:(mb + 1) * SL * 2],
                        in_=kst.bitcast(U8))
                for mb in range(MB):
                    ems.append(lambda mb=mb: e_kp(mb))

                def e_vbegin():
                    vt_t = stream.tile([128, KB, SL], BF16, tag="in3",
                                       name="vt_t")
                    nc.sync.dma_start(out=vt_t, in_=vT_p)
                    st8["vt"] = vt_t
                    for wh in range(2):
                        w = wqp.tile([128, KB, H // 2], BF16, tag="w",
                                     name=f"wv{wh}")
                        nc.sync.dma_start(
                            out=w, in_=Wv_p[:, :, wh * 512:(wh + 1) * 512])
                        st8[f"wv{wh}"] = w
                ems.append(e_vbegin)

                def e_vp(st, wh):
                    # token-major: out [128 tok, 512 ho] ; bias + ones col
                    # fused into the staging copy
                    ps = psA.tile([128, 512], F32, tag="psa",
                                  name=f"vp{st}{wh}")
                    for kb in range(KB):
                        nc.tensor.matmul(
                            ps, st8["vt"][:, kb, st * 128:(st + 1) * 128],
                            st8[f"wv{wh}"][:, kb, :], start=(kb == 0),
                            stop=(kb == KB - 1))
                    hsl = slice(wh * 8, (wh + 1) * 8)
                    vst = stgp.tile([128, 8, HD + 1], BF16, tag="vst",
                                    name="vst")
                    nc.vector.tensor_add(
                        vst[:, :, 0:HD],
                        ps.rearrange("p (hh d) -> p hh d", d=HD),
                        bvp_sb[:, hsl, 0:HD])
                    nc.vector.tensor_copy(vst[:, :, HD:HD + 1],
                                          bvp_sb[:, hsl, HD:HD + 1])
                    off = KPW + st * VW + wh * (VW // 2)
                    nc.sync.dma_start(
                        out=st8["agi"][:, off:off + VW // 2],
                        in_=vst.rearrange("p h c -> p (h c)").bitcast(U8))
                for st in range(KTL):
                    for wh in range(2):
                        ems.append(lambda st=st, wh=wh: e_vp(st, wh))

                def e_ag():
                    if sim:
                        for r in range(CPG):
                            nc.sync.dma_start(out=st8["ago"][r],
                                              in_=st8["agi"])
                    else:
                        nc.gpsimd.collective_compute(
                            "AllGather", mybir.AluOpType.bypass,
                            ins=[st8["agi"].opt()], outs=[st8["ago"].opt()],
                            replica_groups=GROUPS)
                ems.append(e_ag)

                def e_qbegin():
                    qt_t = stream.tile([128, KB, SQ], BF16, tag="in3",
                                       name="qt_t")
                    nc.sync.dma_start(out=qt_t, in_=qT_p)
                    st8["qt"] = qt_t
                    for wh in range(2):
                        w = wqp.tile([128, KB, H // 2], BF16, tag="w",
                                     name=f"wq{wh}")
                        nc.sync.dma_start(
                            out=w, in_=Wq_p[:, :, wh * 512:(wh + 1) * 512])
                        st8[f"wq{wh}"] = w
                ems.append(e_qbegin)

                def e_qp(mb):
                    wh, m = mb // 4, mb % 4
                    ps = psA.tile([128, SQ], F32, tag="psa", name=f"qp{mb}")
                    for kb in range(KB):
                        nc.tensor.matmul(
                            ps, st8[f"wq{wh}"][:, kb, m * 128:(m + 1) * 128],
                            st8["qt"][:, kb, :], start=(kb == 0),
                            stop=(kb == KB - 1))
                    nc.vector.tensor_scalar_add(
                        qpTs[par][:, mb, :], ps, bq_sb[:, mb:mb + 1])
                for mb in range(MB):
                    ems.append(lambda mb=mb: e_qp(mb))

                return ems, st8

            pending, pstate = emit_iter_inputs(0)
            for it in range(sreps):
                par = it % 2
                kpT, vp, qpT = kpTs[par], vps[par], qpTs[par]
                for e in pending:
                    e()
                ago = pstate["ago"]

                # ---- scatter gathered kp/vp into attention layout ----
                for r in range(CPG):
                    nc.sync.dma_start(
                        out=kpT[:, :, r * SL:(r + 1) * SL],
                        in_=ago[r, :, 0:KPW].bitcast(BF16).rearrange(
                            "p (m t) -> p m t", t=SL))
                    nc.sync.dma_start(
                        out=vp[:, r * KTL:(r + 1) * KTL],
                        in_=ago[r, :, KPW:AGW].bitcast(BF16).rearrange(
                            "p (k h c) -> p k h c", h=NH, c=HD + 1))

                if it + 1 < sreps:
                    filler, pstate = emit_iter_inputs(it + 1)
                else:
                    filler, pstate = [], None
                pending = filler  # drained via attention interleave or next it

                if upto < 2:
                    nc.sync.dma_start(out=outT_p[:, 0, :],
                                      in_=kpT[:, 0, 0:SQ])
                    nc.sync.dma_start(out=outT_p[:, 1, :],
                                      in_=qpT[:, 0, :])
                    nc.gpsimd.dma_start(out=outT_p[:, 2, 0:65],
                                        in_=vp[:, 0, 0, :])
                    continue

                # ---- attention; iteration it+1's projections fill the
                # exp-paced PE gaps ----
                n_steps = 2 * 4 * (KTB // 2) * 2
                every = max(1, n_steps // max(1, len(filler)))
                fi = 0
                step = 0
                for hf in range(2):
                    for pr in range(4):
                        mb = hf * 4 + pr
                        accs = [psacc.tile([128, SQ], F32, tag="acc",
                                           name=f"acc{j}") for j in range(2)]
                        for cc in range(KTB // 2):
                            for j in range(2):
                                p1 = ps2b.tile([128, 2, SQ], F32, tag="sc",
                                               name=f"sc{j}")
                                for i in range(2):
                                    ktb = cc * 2 + i
                                    nc.tensor.matmul(
                                        p1[:, i, :],
                                        kpT[64 * j:64 * j + 64, mb,
                                            ktb * 128:(ktb + 1) * 128],
                                        qpT[64 * j:64 * j + 64, mb, :],
                                        start=True, stop=True)
                                et = exps.tile([128, 2, SQ], BF16, tag="et",
                                               name=f"et{j}")
                                nc.scalar.activation(
                                    out=et, in_=p1,
                                    func=mybir.ActivationFunctionType.Exp,
                                    bias=ebias[:, 0:1])
                                h = 2 * mb + j
                                for i in range(2):
                                    ktb = cc * 2 + i
                                    nc.tensor.matmul(
                                        accs[j][0:HD + 1, :],
                                        vp[:, ktb, h, :], et[:, i, :],
                                        start=(ktb == 0),
                                        stop=(ktb == KTB - 1))
                                step += 1
                                if step % every == 0 and fi < len(filler):
                                    filler[fi]()
                                    fi += 1
                        for j in range(2):
                            zr = zrp.tile([1, SQ], F32, tag="zr")
                            with nc.allow_low_precision(
                                    reason="softmax 1/Z; DVE mul"):
                                nc.vector.reciprocal(
                                    zr, accs[j][HD:HD + 1, :])
                            zb = zrp.tile([64, SQ], F32, tag="zb")
                            nc.gpsimd.partition_broadcast(zb, zr)
                            nc.vector.tensor_mul(
                                ctxnT[64 * j:64 * j + 64, mb, :],
                                accs[j][0:HD, :], zb)
                while fi < len(filler):
                    filler[fi]()
                    fi += 1
                pending = []

                if upto < 3:
                    nc.sync.dma_start(out=outT_p[:, 0, :],
                                      in_=ctxnT[:, 0, :])
                    continue

                # ---- output projection (bf16, bias fused) ----
                for ob in range(MB):
                    wo_t = wop.tile([128, KB, 128], BF16, tag="wo_t")
                    nc.sync.dma_start(
                        out=wo_t, in_=Wo_p[:, :, ob * 128:(ob + 1) * 128])
                    po = psA.tile([128, SQ], F32, tag="psa", name="po")
                    for mb in range(MB):
                        nc.tensor.matmul(po, wo_t[:, mb, :],
                                         ctxnT[:, mb, :],
                                         start=(mb == 0), stop=(mb == MB - 1))
                    ot = stgp.tile([128, SQ], BF16, tag="kst", name="ot")
                    nc.vector.tensor_scalar_add(ot, po,
                                                bo_sb[:, ob:ob + 1])
                    nc.sync.dma_start(out=outT_p[:, ob, :], in_=ot)

    nc.compile()
    return nc


def shard_inputs(q, k, v, Wq, bq, Wk, bk, Wv, bv, Wo, bo):
    """Host-side sharding: per-core input dicts."""
    import ml_dtypes
    bf16 = ml_dtypes.bfloat16
    scale = np.float32(1.0 / np.sqrt(HD))
    c32 = lambda a: np.ascontiguousarray(a, dtype=np.float32)
    cbf = lambda a: np.ascontiguousarray(np.asarray(a, dtype=np.float32),
                                         dtype=bf16)
    Wq_b = cbf(c32(Wq) * scale)
    Wk_b, Wv_b, Wo_b = cbf(Wk), cbf(Wv), cbf(Wo)
    bqT = np.ascontiguousarray((c32(bq) * scale).reshape(MB, 128).T)
    bkT = np.ascontiguousarray(c32(bk).reshape(MB, 128).T)
    boT = np.ascontiguousarray(c32(bo).reshape(MB, 128).T)
    bvp = np.ones((128, NH, HD + 1), dtype=bf16)
    bvp[:, :, 0:HD] = cbf(bv).reshape(1, NH, HD)
    in_maps = []
    for c in range(N_CORES):
        b, r = c // CPG, c % CPG
        sl = slice(r * SQ, (r + 1) * SQ)
        in_maps.append({
            "qT": cbf(q[b, sl, :].T), "kT": cbf(k[b, sl, :].T),
            "vT": cbf(v[b, sl, :].T),
            "Wq": Wq_b, "Wk": Wk_b, "Wv": Wv_b, "Wo": Wo_b,
            "bqT": bqT, "bkT": bkT, "boT": boT, "bvp": bvp,
        })
    return in_maps


_NC_CACHE = {}


def get_nc():
    if "nc" not in _NC_CACHE:
        _NC_CACHE["nc"] = build_nc()
    return _NC_CACHE["nc"]


def kernel(q, k, v, Wq, bq, Wk, bk, Wv, bv, Wo, bo):
    q, k, v = np.asarray(q), np.asarray(k), np.asarray(v)
    in_maps = shard_inputs(q, k, v, Wq, bq, Wk, bk, Wv, bv, Wo, bo)
    nc = get_nc()
    res = run_bass_kernel_spmd(nc, in_maps, core_ids=list(range(N_CORES)))
    out = np.empty((B, S, H), dtype=np.float32)
    for c in range(N_CORES):
        b, r0 = c // CPG, (c % CPG) * SQ
        out[b, r0:r0 + SQ, :] = np.asarray(
            res.results[c]["outT"], dtype=np.float32).T
    return out
